# revision 54
# baseline (speedup 1.0000x reference)
"""BailingMoE block on 8 Trainium2 NeuronCores.

Sharding:
  - Attention: data-parallel over tokens (core i owns tokens [128i, 128(i+1))).
    k/v heads are computed per-chunk, rope'd, transposed, then AllGathered (bf16).
  - Router: fp32 per-chunk (top-2 flip-safe), combine-weight matrix AllGathered.
  - MoE: expert-parallel. Core e computes its expert densely over all 1024
    tokens, scaled per-token by the routed weight (0 when unrouted), plus a
    1/8 column shard of the shared expert; partials are ReduceScattered (bf16)
    back to token chunks.
  - Big matmuls in bf16 (fp32 PSUM accumulation); residual/router/softmax fp32.

Single uniform NEFF on all 8 cores; per-core behavior differs only through
input data (weight shards, masks, rope tables, expert selector).
"""

import numpy as np

import concourse.bass as bass
import concourse.bacc as bacc
import concourse.mybir as mybir
import concourse.tile as tile
from concourse.bass_utils import run_bass_kernel_spmd
from concourse.masks import make_identity

F32 = mybir.dt.float32
BF16 = mybir.dt.bfloat16
F8 = mybir.dt.float8e4
AF = mybir.ActivationFunctionType
ALU = mybir.AluOpType
AX = mybir.AxisListType
PM = mybir.MatmulPerfMode

N_CORES = 8
T = 1024          # tokens
TC = 128          # tokens per core chunk
H = 2048          # hidden
NH = 16           # q heads
NKV = 4           # kv heads
DH = 128          # head dim
E = 8             # experts
I = 1024          # moe intermediate
IS = 1024         # shared intermediate
ISC = IS // N_CORES   # shared shard cols per core
QKV = (NH + 2 * NKV) * DH  # 3072
KH = H // 128     # 16 k-tiles over hidden
EPS = 1e-6
SCALE = DH ** -0.5
NEG = -1e9
WS = 32.0         # fp8 weight pre-scale (wgu/wd)
AS = 4.0          # fp8 act pre-scale

_cache = {}


def _bc(ap, n, axis=1):
    """Insert a broadcast (step 0, count n) free dim into an AP at `axis`."""
    a = [list(p) for p in ap.ap]
    a.insert(axis, [0, n])
    return bass.AP(tensor=ap.tensor, offset=ap.offset, ap=a)


def build_nc():
    nc = bacc.Bacc("TRN2", target_bir_lowering=False, num_devices=N_CORES)

    # ---- I/O ----
    x_chunk = nc.dram_tensor("x_chunk", [TC, H], F32, kind="ExternalInput")
    wqkv_bf = nc.dram_tensor("wqkv_bf", [H, QKV], BF16, kind="ExternalInput")
    wo_bf = nc.dram_tensor("wo_bf", [NH * DH, H], BF16, kind="ExternalInput")
    # fp8 expert weights, DoubleRow-packed:
    #   wgu_f8[mi, r, j*256 + i*128 + c] = WS * wgu[(2j+i)*128 + r, mi*128 + c]
    #   wd_f8[j, r, i*2048 + c] = WS * wd[(2j+i)*128 + r, c]
    wgu_f8 = nc.dram_tensor("wgu_f8", [2 * I // 128, 128, 2048], F8,
                            kind="ExternalInput")
    wd_f8 = nc.dram_tensor("wd_f8", [I // 256, 128, 4096], F8,
                           kind="ExternalInput")
    wsgu_bf = nc.dram_tensor("wsgu_bf", [H, 2 * IS], BF16, kind="ExternalInput")
    wsd_bf = nc.dram_tensor("wsd_bf", [IS, H], BF16, kind="ExternalInput")
    wrT = nc.dram_tensor("wrT", [H, E], F32, kind="ExternalInput")
    rope_q = nc.dram_tensor("rope_q", [TC, 4, DH // 2], F32, kind="ExternalInput")
    rope_k = nc.dram_tensor("rope_k", [TC, 4, DH // 2], F32, kind="ExternalInput")
    mask_in = nc.dram_tensor("mask_in", [T, TC], F32, kind="ExternalInput")
    esel = nc.dram_tensor("esel", [1, E], F32, kind="ExternalInput")
    out_chunk = nc.dram_tensor("out_chunk", [TC, H], F32, kind="ExternalOutput")

    rg = [list(range(N_CORES))]

    with tile.TileContext(nc) as tc:
        with tc.tile_pool(name="dram", bufs=1, space="DRAM") as dram, \
             tc.tile_pool(name="const", bufs=1) as const, \
             tc.tile_pool(name="mid", bufs=1) as mid, \
             tc.tile_pool(name="sb", bufs=2) as sb, \
             tc.tile_pool(name="ps_big", bufs=5, space="PSUM") as ps_big, \
             tc.tile_pool(name="ps_sm", bufs=2, space="PSUM") as ps_sm:

            # ---- DRAM collective buffers ----
            KVSZ = NKV * DH * TC * 2  # kT block + v block (elements)
            kv_in = dram.tile([KVSZ], BF16)
            kv_out = dram.tile([N_CORES * KVSZ], BF16, addr_space="Shared")
            h2T_in = dram.tile([H, TC], F8)
            h2T_out = dram.tile([N_CORES * H, TC], F8, addr_space="Shared")
            w_in = dram.tile([TC, E], F32)
            w_out = dram.tile([T, E], F32, addr_space="Shared")
            rs_in = dram.tile([T, H], BF16)
            rs_out = dram.tile([TC, H], BF16)
            wrow_dram = dram.tile([T, 1], F32)

            # ---- constants ----
            ident_bf = const.tile([128, 128], BF16)
            make_identity(nc, ident_bf)
            ident_f = const.tile([128, 128], F32)
            make_identity(nc, ident_f)
            ident_f8 = const.tile([128, 128], F8)
            make_identity(nc, ident_f8)

            eps_sb = const.tile([128, 1], F32)
            nc.vector.memset(eps_sb, EPS)
            esel_sb = const.tile([128, E], F32)
            nc.sync.dma_start(
                out=esel_sb,
                in_=bass.AP(tensor=esel, offset=0, ap=[[0, 128], [1, E]]))
            mask_sb = const.tile([128, N_CORES, TC], F32)
            nc.sync.dma_start(
                out=mask_sb,
                in_=bass.AP(tensor=mask_in, offset=0,
                            ap=[[TC, 128], [128 * TC, N_CORES], [1, TC]]))
            wrT_sb = const.tile([128, KH, E], F32)
            nc.sync.dma_start(
                out=wrT_sb,
                in_=bass.AP(tensor=wrT, offset=0,
                            ap=[[E, 128], [128 * E, KH], [1, E]]))

            # ---- persistent (cross-phase) tiles ----
            x2_sb = mid.tile([TC, H], F32)
            # act pairs for DoubleRow: plane i of pair j is i-tile 2j+i
            act_pr = [mid.tile([128, 2, T], F8, tag=f"act{j}", name=f"act{j}")
                      for j in range(I // 256)]
            w_bcast = mid.tile([128, T], F32)
            shared_sb = mid.tile([TC, H], F32)
            h2Ts = [mid.tile([128, TC], BF16, tag=f"h2Ts{j}", name=f"h2Ts{j}")
                    for j in range(KH)]
            h2f8T = [mid.tile([128, TC], F8, tag=f"h2f8T{j}", name=f"h2f8T{j}")
                     for j in range(KH)]

            def rms_scale(xt, d, tag):
                sq = sb.tile([TC, H], F32, tag="rmssq", bufs=1)
                nc.vector.tensor_mul(sq[:, :d], xt, xt)
                red = sb.tile([TC, 1], F32, tag=f"rred{tag}")
                nc.vector.tensor_reduce(red, sq[:, :d], axis=AX.X, op=ALU.add)
                nc.scalar.activation(red, red, AF.Sqrt, bias=eps_sb[:TC], scale=1.0 / d)
                nc.vector.reciprocal(red, red)
                return red

            # ================= ATTENTION PHASE =================
            with tc.tile_pool(name="apool", bufs=2) as ap_, \
                 tc.tile_pool(name="wstream", bufs=3) as wstream:
                x_sb = ap_.tile([TC, H], F32, tag="x_sb", bufs=1)
                nc.sync.dma_start(out=x_sb, in_=x_chunk[:, :])
                rope_q_sb = ap_.tile([TC, 4, DH // 2], F32, tag="ropeq", bufs=1)
                nc.sync.dma_start(out=rope_q_sb, in_=rope_q[:, :, :])
                rope_k_sb = ap_.tile([TC, 4, DH // 2], F32, tag="ropek", bufs=1)
                nc.sync.dma_start(out=rope_k_sb, in_=rope_k[:, :, :])

                rs1 = rms_scale(x_sb, H, "1")
                h1_bf = ap_.tile([TC, H], BF16, tag="h1bf", bufs=1)
                nc.vector.tensor_scalar_mul(h1_bf, x_sb, rs1)

                # h1T via PE transpose
                h1T = []
                for j in range(KH):
                    pt = ps_sm.tile([128, 128], BF16, tag="pstb")
                    nc.tensor.transpose(pt, h1_bf[:, j * 128:(j + 1) * 128], ident_bf)
                    t_ = ap_.tile([128, TC], BF16, tag=f"h1T{j}", bufs=1)
                    nc.vector.tensor_copy(t_, pt)
                    h1T.append(t_)

                # qkv = h1 @ wqkv  -> [TC, 3072] fp32
                qkv_f = ap_.tile([TC, QKV], F32, tag="qkvf", bufs=1)
                for n in (4, 5):
                    wk = wstream.tile([128, 8, 512], BF16, tag="wst")
                    wk2 = wstream.tile([128, 8, 512], BF16, tag="wst")
                    nc.sync.dma_start(
                        out=wk,
                        in_=bass.AP(tensor=wqkv_bf, offset=n * 512,
                                    ap=[[QKV, 128], [128 * QKV, 8], [1, 512]]))
                    nc.scalar.dma_start(
                        out=wk2,
                        in_=bass.AP(tensor=wqkv_bf, offset=8 * 128 * QKV + n * 512,
                                    ap=[[QKV, 128], [128 * QKV, 8], [1, 512]]))
                    pq = ps_big.tile([TC, 512], F32, tag="mm512")
                    for k in range(KH):
                        src = wk[:, k, :] if k < 8 else wk2[:, k - 8, :]
                        nc.tensor.matmul(pq, h1T[k], src,
                                         start=(k == 0), stop=(k == KH - 1))
                    nc.vector.tensor_copy(qkv_f[:, n * 512:(n + 1) * 512], pq)

                q3 = qkv_f[:, 0:NH * DH].rearrange("p (h d) -> p h d", h=NH)
                k3 = qkv_f[:, NH * DH:(NH + NKV) * DH].rearrange(
                    "p (h d) -> p h d", h=NKV)
                v2d = qkv_f[:, (NH + NKV) * DH:]

                # per-head rmsnorm on q, k (in fp32, in place)
                def qk_norm(x3, nh, tag):
                    sq = sb.tile([TC, H], F32, tag="rmssq", bufs=1)
                    x2dv = x3.rearrange("p h d -> p (h d)")
                    nc.vector.tensor_mul(sq[:, :nh * DH], x2dv, x2dv)
                    red = ap_.tile([TC, nh, 1], F32, tag=f"qred{tag}")
                    nc.vector.tensor_reduce(
                        red, sq[:, :nh * DH].rearrange("p (h d) -> p h d", h=nh),
                        axis=AX.X, op=ALU.add)
                    nc.scalar.activation(red.rearrange("p h one -> p (h one)"), red.rearrange("p h one -> p (h one)"), AF.Sqrt, bias=eps_sb[:TC], scale=1.0 / DH)
                    nc.vector.reciprocal(
                        red.rearrange("p h one -> p (h one)"),
                        red.rearrange("p h one -> p (h one)"))
                    for h in range(nh):
                        nc.vector.tensor_scalar_mul(
                            x3[:, h, :], x3[:, h, :], red[:, h, :])

                qk_norm(k3, NKV, "k")

                # rope (+ qk-norm weight folded into tables), cast into qkv_bf
                qkv_bf = ap_.tile([TC, QKV], BF16, tag="qkvbf", bufs=1)
                qbf3 = qkv_bf[:, 0:NH * DH].rearrange("p (h d) -> p h d", h=NH)
                kbf3 = qkv_bf[:, NH * DH:(NH + NKV) * DH].rearrange(
                    "p (h d) -> p h d", h=NKV)

                def rope(x3, obf3, nh, tab):
                    c1 = _bc(tab[:, 0, :], nh)
                    s1 = _bc(tab[:, 1, :], nh)
                    c2 = _bc(tab[:, 2, :], nh)
                    s2 = _bc(tab[:, 3, :], nh)
                    x1 = x3[:, :, 0:DH // 2]
                    x2 = x3[:, :, DH // 2:DH]
                    t1 = ap_.tile([TC, NH, DH // 2], F32, tag="rp1", bufs=1)
                    tn = ap_.tile([TC, NH, DH // 2], F32, tag="rpn", bufs=1)
                    t1v = t1[:, :nh, :]
                    tnv = tn[:, :nh, :]
                    nc.vector.tensor_mul(t1v, x1, c1)
                    nc.vector.tensor_mul(tnv, x2, s1)
                    nc.vector.tensor_sub(t1v, t1v, tnv)
                    nc.vector.tensor_copy(obf3[:, :, 0:DH // 2], t1v)
                    nc.vector.tensor_mul(t1v, x2, c2)
                    nc.vector.tensor_mul(tnv, x1, s2)
                    nc.vector.tensor_add(t1v, t1v, tnv)
                    nc.vector.tensor_copy(obf3[:, :, DH // 2:DH], t1v)

                rope(k3, kbf3, NKV, rope_k_sb)
                nc.vector.tensor_copy(qkv_bf[:, (NH + NKV) * DH:], v2d)

                # transpose k heads -> kT_in (DRAM)
                for g in range(NKV):
                    pt = ps_sm.tile([128, 128], BF16, tag="pstb")
                    nc.tensor.transpose(
                        pt, qkv_bf[:, (NH + g) * DH:(NH + g + 1) * DH],
                        ident_bf)
                    t_ = ap_.tile([DH, TC], BF16, tag="kTs")
                    nc.vector.tensor_copy(t_, pt)
                    nc.sync.dma_start(
                        out=bass.AP(tensor=kv_in.tensor,
                                    offset=kv_in.offset + g * DH * TC,
                                    ap=[[TC, DH], [1, TC]]),
                        in_=t_)
                nc.sync.dma_start(
                    out=bass.AP(tensor=kv_in.tensor,
                                offset=kv_in.offset + NKV * DH * TC,
                                ap=[[NKV * DH, TC], [1, NKV * DH]]),
                    in_=qkv_bf[:, (NH + NKV) * DH:])
                nc.gpsimd.collective_compute(
                    "AllGather", ALU.bypass, replica_groups=rg,
                    ins=[kv_in.opt()], outs=[kv_out.opt()])

                # q columns of the projection (overlaps the kv AllGather)
                for n in range(4):
                    wk = wstream.tile([128, 8, 512], BF16, tag="wst")
                    wk2 = wstream.tile([128, 8, 512], BF16, tag="wst")
                    nc.sync.dma_start(
                        out=wk,
                        in_=bass.AP(tensor=wqkv_bf, offset=n * 512,
                                    ap=[[QKV, 128], [128 * QKV, 8], [1, 512]]))
                    nc.scalar.dma_start(
                        out=wk2,
                        in_=bass.AP(tensor=wqkv_bf, offset=8 * 128 * QKV + n * 512,
                                    ap=[[QKV, 128], [128 * QKV, 8], [1, 512]]))
                    pq = ps_big.tile([TC, 512], F32, tag="mm512")
                    for k in range(KH):
                        src = wk[:, k, :] if k < 8 else wk2[:, k - 8, :]
                        nc.tensor.matmul(pq, h1T[k], src,
                                         start=(k == 0), stop=(k == KH - 1))
                    nc.vector.tensor_copy(qkv_f[:, n * 512:(n + 1) * 512], pq)
                qk_norm(q3, NH, "q")
                rope(q3, qbf3, NH, rope_q_sb)
                # transpose q heads -> qT
                qT = []
                for h in range(NH):
                    pt = ps_sm.tile([128, 128], BF16, tag="pstb")
                    nc.tensor.transpose(
                        pt, qkv_bf[:, h * DH:(h + 1) * DH], ident_bf)
                    t_ = ap_.tile([DH, TC], BF16, tag=f"qT{h}", bufs=1)
                    nc.vector.tensor_copy(t_, pt)
                    qT.append(t_)

                # attention per q head; kT/v loaded per kv-head group.
                # software-pipelined: scores/exp for head h are emitted before
                # the ctx-tail of head h-1 so the in-order PE queue overlaps
                # PE scores with the DVE mask-add / Act exp of the prior head.
                ctxT = [None] * NH

                def load_group(g):
                    kT = ap_.tile([DH, N_CORES, TC], BF16, tag="kTg", bufs=2)
                    nc.scalar.dma_start(
                        out=kT,
                        in_=bass.AP(
                            tensor=kv_out.tensor,
                            offset=kv_out.offset + g * DH * TC,
                            ap=[[TC, DH], [KVSZ, N_CORES], [1, TC]]))
                    vg = [ap_.tile([TC, DH + 1], BF16, tag=f"vg{j}", bufs=2,
                                   name=f"vg{j}")
                          for j in range(N_CORES)]
                    for j in range(N_CORES):
                        nc.vector.memset(vg[j][:, DH:DH + 1], 1.0)
                        nc.sync.dma_start(
                            out=vg[j][:, 0:DH],
                            in_=bass.AP(
                                tensor=kv_out.tensor,
                                offset=kv_out.offset + j * KVSZ
                                + NKV * DH * TC + g * DH,
                                ap=[[NKV * DH, TC], [1, DH]]))
                    return kT, vg

                def head_front(h, kT, vg):
                    probs = ap_.tile([128, N_CORES, TC], F32, tag="probs",
                                     bufs=2)
                    for half in range(2):
                        ps = ps_big.tile([TC, 512], F32, tag="mm512")
                        for jj in range(4):
                            j = half * 4 + jj
                            nc.tensor.matmul(
                                ps[:, jj * TC:(jj + 1) * TC],
                                kT[:, j, :], qT[h], start=True, stop=True)
                        nc.vector.tensor_add(
                            probs.rearrange("p j q -> p (j q)")
                            [:, half * 512:(half + 1) * 512],
                            ps,
                            mask_sb.rearrange("p j q -> p (j q)")
                            [:, half * 512:(half + 1) * 512])
                    pflat = probs.rearrange("p j q -> p (j q)")
                    probs_bf = ap_.tile([128, N_CORES, TC], BF16, tag="probsbf",
                                        bufs=2)
                    nc.scalar.activation(
                        probs_bf.rearrange("p j q -> p (j q)"), pflat,
                        AF.Exp, scale=SCALE)
                    return probs_bf

                def head_tail(h, probs_bf, vg):
                    pctx_t = ps_big.tile([TC, 512], F32, tag="mm512")
                    pctx = pctx_t[:, 0:DH + 1]
                    for j in range(N_CORES):
                        nc.tensor.matmul(pctx, probs_bf[:, j, :], vg[j],
                                         start=(j == 0),
                                         stop=(j == N_CORES - 1))
                    rden = sb.tile([TC, 1], F32, tag="rden")
                    nc.vector.reciprocal(rden, pctx[:, DH:DH + 1])
                    ctx_bf = sb.tile([TC, DH], BF16, tag="ctxbf")
                    nc.vector.tensor_scalar_mul(ctx_bf, pctx[:, 0:DH], rden)
                    pt2 = ps_sm.tile([128, 128], BF16, tag="pstb")
                    nc.tensor.transpose(pt2, ctx_bf, ident_bf)
                    t_ = ap_.tile([DH, TC], BF16, tag=f"ctxT{h}", bufs=1)
                    nc.scalar.activation(t_, pt2, AF.Copy)
                    ctxT[h] = t_

                GSZ = NH // NKV
                cur = load_group(0)
                nxt = None
                pend = None
                for h in range(NH):
                    if h % GSZ == 0 and h > 0:
                        cur = nxt
                    if h % GSZ == 1 and h // GSZ < NKV - 1:
                        nxt = load_group(h // GSZ + 1)
                    probs_bf = head_front(h, cur[0], cur[1])
                    if pend is not None:
                        head_tail(*pend)
                    pend = (h, probs_bf, cur[1])
                head_tail(*pend)

                # attn_out = ctx @ wo ; x2 = x + attn_out
                for n in range(H // 512):
                    wk = wstream.tile([128, 8, 512], BF16, tag="wst")
                    wk2 = wstream.tile([128, 8, 512], BF16, tag="wst")
                    nc.sync.dma_start(
                        out=wk,
                        in_=bass.AP(tensor=wo_bf, offset=n * 512,
                                    ap=[[H, 128], [128 * H, 8], [1, 512]]))
                    nc.scalar.dma_start(
                        out=wk2,
                        in_=bass.AP(tensor=wo_bf, offset=8 * 128 * H + n * 512,
                                    ap=[[H, 128], [128 * H, 8], [1, 512]]))
                    po = ps_big.tile([TC, 512], F32, tag="mm512")
                    for k in range(NH * DH // 128):
                        src = wk[:, k, :] if k < 8 else wk2[:, k - 8, :]
                        nc.tensor.matmul(po, ctxT[k], src,
                                         start=(k == 0), stop=(k == KH - 1))
                    nc.vector.tensor_add(x2_sb[:, n * 512:(n + 1) * 512], po,
                                         x_sb[:, n * 512:(n + 1) * 512])

                # ---- h2 (still inside attention pool scope) ----
                rs2 = rms_scale(x2_sb, H, "2")
                h2_f = mid.tile([TC, H], F32, tag="h2f")
                nc.vector.tensor_scalar_mul(h2_f, x2_sb, rs2)
                h2_bf = ap_.tile([TC, H], BF16, tag="h2bf", bufs=1)
                nc.vector.tensor_copy(h2_bf, h2_f)

                for j in range(KH):
                    pt = ps_sm.tile([128, 128], BF16, tag="pstb")
                    nc.tensor.transpose(pt, h2_bf[:, j * 128:(j + 1) * 128], ident_bf)
                    nc.vector.tensor_copy(h2Ts[j], pt)
                    nc.vector.tensor_copy(h2f8T[j], h2Ts[j])
                    nc.sync.dma_start(out=h2T_in[j * 128:(j + 1) * 128, :],
                                      in_=h2f8T[j])

                nc.gpsimd.collective_compute(
                    "AllGather", ALU.bypass, replica_groups=rg,
                    ins=[h2T_in.opt()], outs=[h2T_out.opt()])

            # ================= MOE PHASE =================
            moe_ctx = [tc.tile_pool(name="h2Tp", bufs=1),
                       tc.tile_pool(name="wgup", bufs=6),
                       tc.tile_pool(name="wdp", bufs=1)]
            h2Tp, wgup, wdp = [c.__enter__() for c in moe_ctx]
            # prefetch fp8 wd pair tiles (independent of collectives)
            wd_sb = []
            for j in range(I // 256):
                t_ = wdp.tile([128, 2, H], F8, tag=f"wd{j}", name=f"wd{j}")
                (nc.scalar if j % 2 == 0 else nc.sync).dma_start(
                    out=t_,
                    in_=bass.AP(tensor=wd_f8, offset=j * 128 * 4096,
                                ap=[[4096, 128], [2048, 2], [1, 2048]]))
                wd_sb.append(t_)

            # shared expert on OWN token chunk (no AG dependency - fills
            # the h2T AllGather gap)
            with tc.tile_pool(name="wsp", bufs=3) as wsp, \
                 tc.tile_pool(name="wsdp", bufs=1) as wsdp:
                gus_bf = []
                for n in range(2 * IS // 512):
                    wsg = wsp.tile([128, KH, 512], BF16, tag="wsg")
                    (nc.sync if n % 2 == 0 else nc.scalar).dma_start(
                        out=wsg,
                        in_=bass.AP(tensor=wsgu_bf, offset=n * 512,
                                    ap=[[2 * IS, 128], [128 * 2 * IS, KH],
                                        [1, 512]]))
                    pgu = ps_big.tile([TC, 512], F32, tag="mm512")
                    for k in range(KH):
                        nc.tensor.matmul(pgu, h2Ts[k], wsg[:, k, :],
                                         start=(k == 0), stop=(k == KH - 1))
                    t_ = sb.tile([TC, 512], BF16, tag="gusbf", bufs=4,
                                 name=f"gus{n}")
                    if n < IS // 512:
                        nc.scalar.activation(t_, pgu, AF.Silu)
                    else:
                        nc.vector.tensor_copy(t_, pgu)
                    gus_bf.append(t_)
                # acts_own[t, i] = silu(g)*u ; transpose to [IS, TC]
                actsT = []
                for n in range(IS // 512):
                    nc.vector.tensor_mul(gus_bf[n], gus_bf[n],
                                         gus_bf[n + IS // 512])
                    for jj in range(4):
                        i = n * 4 + jj
                        pt = ps_sm.tile([128, 128], BF16, tag="pstb")
                        nc.tensor.transpose(
                            pt, gus_bf[n][:, jj * 128:(jj + 1) * 128], ident_bf)
                        t_ = sb.tile([128, TC], BF16, tag=f"actsT{i}", bufs=1,
                                     name=f"actsT{i}")
                        nc.vector.tensor_copy(t_, pt)
                        actsT.append(t_)
                # stream wsd row-tiles (i outer) into 4 persistent psums
                pshs = [ps_big.tile([TC, 512], F32, tag="mm512",
                                    name=f"psh{n_}")
                        for n_ in range(H // 512)]
                for i in range(IS // 128):
                    t_ = wsdp.tile([128, H], BF16, tag="wsd", bufs=3)
                    (nc.sync if i % 2 == 0 else nc.scalar).dma_start(
                        out=t_, in_=wsd_bf[i * 128:(i + 1) * 128, :])
                    for n in range(H // 512):
                        nc.tensor.matmul(pshs[n], actsT[i],
                                         t_[:, n * 512:(n + 1) * 512],
                                         start=(i == 0),
                                         stop=(i == IS // 128 - 1))
                for n in range(H // 512):
                    nc.vector.tensor_add(
                        shared_sb[:, n * 512:(n + 1) * 512], pshs[n],
                        x2_sb[:, n * 512:(n + 1) * 512])

            # fp32 router on own chunk (after the shared expert so its PE
            # transposes don't block the in-order PE stream during the AG gap)
            with tc.tile_pool(name="rtp", bufs=2) as rtp:
                pr = ps_big.tile([TC, E], F32, tag="mm512")
                for j in range(KH):
                    pt = ps_sm.tile([128, 128], F32, tag="pstf", bufs=1)
                    nc.tensor.transpose(pt, h2_f[:, j * 128:(j + 1) * 128],
                                        ident_f)
                    t_ = rtp.tile([128, TC], F32, tag="h2T32")
                    nc.vector.tensor_copy(t_, pt)
                    nc.tensor.matmul(pr, t_, wrT_sb[:, j, :],
                                     start=(j == 0), stop=(j == KH - 1))
                probs8 = sb.tile([TC, E], F32, tag="probs8")
                nc.scalar.activation(probs8, pr, AF.Exp, scale=1.0)
                den8 = sb.tile([TC, 1], F32, tag="den8")
                nc.vector.tensor_reduce(den8, probs8, axis=AX.X, op=ALU.add)
                rden8 = sb.tile([TC, 1], F32, tag="rden8")
                nc.vector.reciprocal(rden8, den8)
                nc.vector.tensor_scalar_mul(probs8, probs8, rden8)
                mx8 = sb.tile([TC, 8], F32, tag="mx8")
                nc.vector.max(out=mx8, in_=probs8)
                s12 = sb.tile([TC, 1], F32, tag="s12")
                nc.vector.tensor_add(s12, mx8[:, 0:1], mx8[:, 1:2])
                rs12 = sb.tile([TC, 1], F32, tag="rs12")
                nc.vector.reciprocal(rs12, s12)
                eq1 = sb.tile([TC, E], F32, tag="eq1")
                nc.vector.tensor_scalar(eq1, probs8, mx8[:, 0:1], None,
                                        op0=ALU.is_equal)
                eq2 = sb.tile([TC, E], F32, tag="eq2")
                nc.vector.tensor_scalar(eq2, probs8, mx8[:, 1:2], None,
                                        op0=ALU.is_equal)
                nc.vector.tensor_add(eq1, eq1, eq2)
                wm = sb.tile([TC, E], F32, tag="wm")
                nc.vector.tensor_mul(wm, probs8, eq1)
                nc.vector.tensor_scalar_mul(wm, wm, rs12)
                nc.gpsimd.dma_start(out=w_in[:, :], in_=wm)
                nc.gpsimd.collective_compute(
                    "AllGather", ALU.bypass, replica_groups=rg,
                    ins=[w_in.opt()], outs=[w_out.opt()])
            # keep the router ahead of the (long-stalling) gu matmuls in
            # every engine queue - the list scheduler would otherwise sink it
            tc.no_sync_barrier()

            # own-expert weight column -> broadcast row. gpsimd DMA queue
            # (sync/scalar queues must stay clear for h2P/wgu loads) and a
            # partition-broadcast DMA instead of PE matmuls.
            wall = sb.tile([128, T // TC, E], F32, tag="wall", bufs=1)
            nc.gpsimd.dma_start(
                out=wall,
                in_=bass.AP(tensor=w_out.tensor, offset=w_out.offset,
                            ap=[[E, 128], [TC * E, T // TC], [1, E]]))
            nc.vector.tensor_mul(wall, wall, _bc(esel_sb, T // TC))
            wcol_all = sb.tile([128, T // TC, 1], F32, tag="wcol", bufs=1)
            nc.vector.tensor_reduce(wcol_all, wall, axis=AX.X, op=ALU.add)
            nc.gpsimd.dma_start(
                out=bass.AP(tensor=wrow_dram.tensor, offset=wrow_dram.offset,
                            ap=[[1, 128], [TC, T // TC]]),
                in_=wcol_all.rearrange("p c one -> p (c one)"))
            nc.gpsimd.dma_start(
                out=w_bcast,
                in_=bass.AP(tensor=wrow_dram.tensor, offset=wrow_dram.offset,
                            ap=[[0, 128], [1, T]]))

            if True:
                # gu^T = wgu^T @ h2 (fp8 DoubleRow, K=256 per matmul). All g
                # tiles first: the u drains need w_bcast, which arrives late
                # (behind the w AllGather) and would stall the psum pipeline.
                order = list(range(I // 128)) + \
                    [i_ + I // 128 for i_ in range(I // 128)]

                def load_wk(mi):
                    wk = wgup.tile([128, KH // 2, 2, 128], F8, tag="wgu")
                    dma_eng = nc.sync if mi % 2 == 0 else nc.scalar
                    dma_eng.dma_start(
                        out=wk,
                        in_=bass.AP(tensor=wgu_f8, offset=mi * 128 * 2048,
                                    ap=[[2048, 128], [256, KH // 2], [128, 2],
                                        [1, 128]]))
                    return wk

                # prefetch the first wgu tiles while the AllGather is in flight
                wk_q = [load_wk(order[i]) for i in range(4)]

                # gathered fp8 h2^T pairs: plane i of pair j holds k-tile 2j+i
                h2P = []
                for j in range(KH // 2):
                    t_ = h2Tp.tile([128, 2, T], F8, tag=f"h2P{j}",
                                   name=f"h2P{j}")
                    for pl in range(2):
                        (nc.sync if pl == 0 else nc.scalar).dma_start(
                            out=t_[:, pl, :],
                            in_=bass.AP(
                                tensor=h2T_out.tensor,
                                offset=h2T_out.offset
                                + (2 * j + pl) * 128 * TC,
                                ap=[[TC, 128], [H * TC, N_CORES], [1, TC]]))
                    h2P.append(t_)

                def gu_tile(idx, mi):
                    wk = wk_q.pop(0)
                    if idx + 4 < len(order):
                        wk_q.append(load_wk(order[idx + 4]))
                    dst = sb.tile([128, T], BF16,
                                  tag="gtmp" if mi < I // 128 else "utmp",
                                  bufs=I // 128 if mi < I // 128 else 2)
                    for n in range(T // 512):
                        pg = ps_big.tile([128, 512], F32, tag="mm512")
                        for j in range(KH // 2):
                            nc.tensor.matmul(
                                pg, wk[:, j, :, :],
                                h2P[j][:, :, n * 512:(n + 1) * 512],
                                start=(j == 0), stop=(j == KH // 2 - 1),
                                perf_mode=PM.DoubleRow)
                        if mi < I // 128:
                            # g: psum = WS*g -> silu(g)
                            nc.scalar.activation(
                                dst[:, n * 512:(n + 1) * 512], pg,
                                AF.Silu, scale=1.0 / WS)
                        else:
                            # u: psum = WS*u; w_bcast = (AS/WS)*w -> AS*u*w
                            nc.vector.tensor_mul(
                                dst[:, n * 512:(n + 1) * 512], pg,
                                w_bcast[:, n * 512:(n + 1) * 512])
                    return dst

                g_ts = [gu_tile(i_, i_) for i_ in range(I // 128)]
                for i_ in range(I // 128):
                    u_t = gu_tile(I // 128 + i_, i_ + I // 128)
                    nc.vector.tensor_mul(
                        act_pr[i_ // 2][:, i_ % 2, :], g_ts[i_], u_t)

                # routed partial [T, H] = act^T @ wd (fp8 DoubleRow) -> rs_in
                inv_sw = const.tile([128, 1], F32)
                nc.vector.memset(inv_sw, 1.0 / (AS * WS))
                cnt = 0
                for t in range(T // TC):
                    for n in range(H // 512):
                        pd = ps_big.tile([TC, 512], F32, tag="mm512")
                        for j in range(I // 256):
                            nc.tensor.matmul(
                                pd, act_pr[j][:, :, t * TC:(t + 1) * TC],
                                wd_sb[j][:, :, n * 512:(n + 1) * 512],
                                start=(j == 0), stop=(j == I // 256 - 1),
                                perf_mode=PM.DoubleRow)
                        rt = sb.tile([TC, 512], BF16, tag="rt", bufs=4)
                        if cnt % 2 == 0:
                            nc.scalar.activation(rt, pd, AF.Copy,
                                                 scale=1.0 / (AS * WS))
                        else:
                            nc.vector.tensor_scalar_mul(rt, pd, inv_sw[:TC])
                        (nc.sync if cnt % 2 == 0 else nc.scalar).dma_start(
                            out=rs_in[t * TC:(t + 1) * TC,
                                      n * 512:(n + 1) * 512],
                            in_=rt)
                        cnt += 1
                nc.gpsimd.collective_compute(
                    "ReduceScatter", ALU.add, replica_groups=rg,
                    ins=[rs_in.opt()], outs=[rs_out.opt()])

            for c in reversed(moe_ctx):
                c.__exit__(None, None, None)

            moe_bf = sb.tile([TC, H], BF16, tag="moebf", bufs=1)
            moe_f = sb.tile([TC, H], F32, tag="moef", bufs=1)
            for q in range(2):
                sl = slice(q * 1024, (q + 1) * 1024)
                (nc.sync if q == 0 else nc.scalar).dma_start(
                    out=moe_bf[:, sl], in_=rs_out[:, sl])
                nc.vector.tensor_add(moe_f[:, sl], shared_sb[:, sl],
                                     moe_bf[:, sl])
                (nc.sync if q == 0 else nc.scalar).dma_start(
                    out=out_chunk[:, sl], in_=moe_f[:, sl])

    nc.compile()
    return nc


def _prep_inputs(hidden_states, w_ln1, w_ln2, wqkv, q_norm_w, k_norm_w, wo,
                 w_router, w_gu, w_d, ws_gu, ws_d, positions):
    import ml_dtypes
    bf = ml_dtypes.bfloat16
    f8 = ml_dtypes.float8_e4m3

    x = np.asarray(hidden_states, np.float32).reshape(T, H)
    w_ln1 = np.asarray(w_ln1, np.float32)
    w_ln2 = np.asarray(w_ln2, np.float32)
    wqkv_e = (np.asarray(wqkv, np.float32) * w_ln1[:, None]).astype(bf)
    wo_b = np.asarray(wo, np.float32).astype(bf)
    # fp8 DoubleRow-packed expert weights (see build_nc comments)
    wgu_e = (np.asarray(w_gu, np.float32) * w_ln2[None, :, None] * WS)
    wgu_p = wgu_e.reshape(E, KH // 2, 2, 128, 2 * I // 128, 128) \
        .transpose(0, 4, 3, 1, 2, 5).reshape(E, 2 * I // 128, 128, 2048) \
        .astype(f8)
    wd_e = np.asarray(w_d, np.float32) * WS
    wd_p = wd_e.reshape(E, I // 256, 2, 128, H).transpose(0, 1, 3, 2, 4) \
        .reshape(E, I // 256, 128, 4096).astype(f8)
    wsgu_e = (np.asarray(ws_gu, np.float32) * w_ln2[:, None]).astype(bf)
    wsd_b = np.asarray(ws_d, np.float32).astype(bf)
    wrT_e = np.ascontiguousarray(
        (np.asarray(w_router, np.float32) * w_ln2[None, :]).T.astype(np.float32))

    pos = np.asarray(positions).astype(np.float64)
    inv_freq = 1.0 / (10000.0 ** (np.arange(0, DH, 2, dtype=np.float64) / DH))
    freqs = pos[:, None] * inv_freq[None, :]          # [T, 64]
    cos = np.cos(freqs).astype(np.float32)
    sin = np.sin(freqs).astype(np.float32)
    qw = np.asarray(q_norm_w, np.float32)
    kw = np.asarray(k_norm_w, np.float32)

    def rope_tab(w):
        # [T, 4, 64]: (cos*w[:64], sin*w[64:], cos*w[64:], sin*w[:64])
        return np.ascontiguousarray(
            np.stack([cos * w[None, :64], sin * w[None, 64:],
                      cos * w[None, 64:], sin * w[None, :64]], axis=1), np.float32)

    rq = rope_tab(qw)
    rk = rope_tab(kw)

    kidx = np.arange(T)
    in_maps = []
    for c in range(N_CORES):
        rows = np.arange(c * TC, (c + 1) * TC)
        mask = np.ascontiguousarray(
            np.where(rows[:, None] >= kidx[None, :], 0.0, NEG)
            .astype(np.float32).T)  # [T(tk), TC(tq)]
        es = np.zeros((1, E), np.float32)
        es[0, c] = AS / WS
        in_maps.append({
            "x_chunk": np.ascontiguousarray(x[c * TC:(c + 1) * TC]),
            "wqkv_bf": wqkv_e,
            "wo_bf": wo_b,
            "wgu_f8": np.ascontiguousarray(wgu_p[c]),
            "wd_f8": np.ascontiguousarray(wd_p[c]),
            "wsgu_bf": wsgu_e,
            "wsd_bf": wsd_b,
            "wrT": wrT_e,
            "rope_q": np.ascontiguousarray(rq[c * TC:(c + 1) * TC]),
            "rope_k": np.ascontiguousarray(rk[c * TC:(c + 1) * TC]),
            "mask_in": mask,
            "esel": es,
        })
    return in_maps


def kernel(**inputs):
    import os
    if "nc" not in _cache:
        _cache["nc"] = build_nc()
    nc = _cache["nc"]
    in_maps = _prep_inputs(**inputs)
    trace = bool(int(os.environ.get("KERNEL_TRACE", "0")))
    res = run_bass_kernel_spmd(nc, in_maps, core_ids=list(range(N_CORES)),
                               trace=trace)
    _cache["last_result"] = res
    out = np.concatenate(
        [res.results[c]["out_chunk"] for c in range(N_CORES)], axis=0)
    return out.reshape(1, T, H).astype(np.float32)


if __name__ == "__main__":
    import reference
    inp = {k: np.asarray(v) for k, v in reference.setup_inputs().items()}
    got = kernel(**inp)
    exp = np.asarray(reference.reference(**reference.setup_inputs()))
    denom = np.abs(exp).max()
    err = np.abs(got - exp).max() / denom
    print("abs max:", denom, "rel err:", err)



# revision 57
# speedup vs baseline: 1.0121x; 1.0121x over previous
"""BailingMoE block on 8 Trainium2 NeuronCores.

Sharding:
  - Attention: data-parallel over tokens (core i owns tokens [128i, 128(i+1))).
    k/v heads are computed per-chunk, rope'd, transposed, then AllGathered (bf16).
  - Router: fp32 per-chunk (top-2 flip-safe), combine-weight matrix AllGathered.
  - MoE: expert-parallel. Core e computes its expert densely over all 1024
    tokens, scaled per-token by the routed weight (0 when unrouted), plus a
    1/8 column shard of the shared expert; partials are ReduceScattered (bf16)
    back to token chunks.
  - Big matmuls in bf16 (fp32 PSUM accumulation); residual/router/softmax fp32.

Single uniform NEFF on all 8 cores; per-core behavior differs only through
input data (weight shards, masks, rope tables, expert selector).
"""

import numpy as np

import concourse.bass as bass
import concourse.bacc as bacc
import concourse.mybir as mybir
import concourse.tile as tile
from concourse.bass_utils import run_bass_kernel_spmd
from concourse.masks import make_identity

F32 = mybir.dt.float32
BF16 = mybir.dt.bfloat16
F8 = mybir.dt.float8e4
AF = mybir.ActivationFunctionType
ALU = mybir.AluOpType
AX = mybir.AxisListType
PM = mybir.MatmulPerfMode

N_CORES = 8
T = 1024          # tokens
TC = 128          # tokens per core chunk
H = 2048          # hidden
NH = 16           # q heads
NKV = 4           # kv heads
DH = 128          # head dim
E = 8             # experts
I = 1024          # moe intermediate
IS = 1024         # shared intermediate
ISC = IS // N_CORES   # shared shard cols per core
QKV = (NH + 2 * NKV) * DH  # 3072
KH = H // 128     # 16 k-tiles over hidden
EPS = 1e-6
SCALE = DH ** -0.5
NEG = -1e9
WS = 32.0         # fp8 weight pre-scale (wgu/wd)
AS = 4.0          # fp8 act pre-scale

_cache = {}


def _bc(ap, n, axis=1):
    """Insert a broadcast (step 0, count n) free dim into an AP at `axis`."""
    a = [list(p) for p in ap.ap]
    a.insert(axis, [0, n])
    return bass.AP(tensor=ap.tensor, offset=ap.offset, ap=a)


def build_nc():
    nc = bacc.Bacc("TRN2", target_bir_lowering=False, num_devices=N_CORES)

    # ---- I/O ----
    x_chunk = nc.dram_tensor("x_chunk", [TC, H], F32, kind="ExternalInput")
    wqkv_bf = nc.dram_tensor("wqkv_bf", [H, QKV], BF16, kind="ExternalInput")
    wo_bf = nc.dram_tensor("wo_bf", [NH * DH, H], BF16, kind="ExternalInput")
    # fp8 expert weights, DoubleRow-packed:
    #   wgu_f8[mi, r, j*256 + i*128 + c] = WS * wgu[(2j+i)*128 + r, mi*128 + c]
    #   wd_f8[j, r, i*2048 + c] = WS * wd[(2j+i)*128 + r, c]
    wgu_f8 = nc.dram_tensor("wgu_f8", [2 * I // 128, 128, 2048], F8,
                            kind="ExternalInput")
    wd_f8 = nc.dram_tensor("wd_f8", [I // 256, 128, 4096], F8,
                           kind="ExternalInput")
    wsgu_bf = nc.dram_tensor("wsgu_bf", [H, 2 * IS], F8, kind="ExternalInput")
    wsd_bf = nc.dram_tensor("wsd_bf", [IS, H], F8, kind="ExternalInput")
    wrT = nc.dram_tensor("wrT", [H, E], F32, kind="ExternalInput")
    rope_q = nc.dram_tensor("rope_q", [TC, 4, DH // 2], F32, kind="ExternalInput")
    rope_k = nc.dram_tensor("rope_k", [TC, 4, DH // 2], F32, kind="ExternalInput")
    mask_in = nc.dram_tensor("mask_in", [T, TC], F32, kind="ExternalInput")
    esel = nc.dram_tensor("esel", [1, E], F32, kind="ExternalInput")
    out_chunk = nc.dram_tensor("out_chunk", [TC, H], F32, kind="ExternalOutput")

    rg = [list(range(N_CORES))]

    with tile.TileContext(nc) as tc:
        with tc.tile_pool(name="dram", bufs=1, space="DRAM") as dram, \
             tc.tile_pool(name="const", bufs=1) as const, \
             tc.tile_pool(name="mid", bufs=1) as mid, \
             tc.tile_pool(name="sb", bufs=2) as sb, \
             tc.tile_pool(name="ps_big", bufs=5, space="PSUM") as ps_big, \
             tc.tile_pool(name="ps_sm", bufs=2, space="PSUM") as ps_sm:

            # ---- DRAM collective buffers ----
            KVSZ = NKV * DH * TC * 2  # kT block + v block (elements)
            kv_in = dram.tile([KVSZ], BF16)
            kv_out = dram.tile([N_CORES * KVSZ], BF16, addr_space="Shared")
            h2T_in = dram.tile([H, TC], F8)
            h2T_out = dram.tile([N_CORES * H, TC], F8, addr_space="Shared")
            w_in = dram.tile([TC, E], F32)
            w_out = dram.tile([T, E], F32, addr_space="Shared")
            rs_in = dram.tile([T, H], BF16)
            rs_out = dram.tile([TC, H], BF16)
            wrow_dram = dram.tile([T, 1], F32)

            # ---- constants ----
            ident_bf = const.tile([128, 128], BF16)
            make_identity(nc, ident_bf)
            ident_f = const.tile([128, 128], F32)
            make_identity(nc, ident_f)
            ident_f8 = const.tile([128, 128], F8)
            make_identity(nc, ident_f8)

            eps_sb = const.tile([128, 1], F32)
            nc.vector.memset(eps_sb, EPS)
            esel_sb = const.tile([128, E], F32)
            nc.sync.dma_start(
                out=esel_sb,
                in_=bass.AP(tensor=esel, offset=0, ap=[[0, 128], [1, E]]))
            mask_sb = const.tile([128, N_CORES, TC], F32)
            nc.sync.dma_start(
                out=mask_sb,
                in_=bass.AP(tensor=mask_in, offset=0,
                            ap=[[TC, 128], [128 * TC, N_CORES], [1, TC]]))
            wrT_sb = const.tile([128, KH, E], F32)
            nc.sync.dma_start(
                out=wrT_sb,
                in_=bass.AP(tensor=wrT, offset=0,
                            ap=[[E, 128], [128 * E, KH], [1, E]]))

            # ---- persistent (cross-phase) tiles ----
            x2_sb = mid.tile([TC, H], F32)
            # act pairs for DoubleRow: plane i of pair j is i-tile 2j+i
            act_pr = [mid.tile([128, 2, T], F8, tag=f"act{j}", name=f"act{j}")
                      for j in range(I // 256)]
            w_bcast = mid.tile([128, T], F32)
            shared_sb = mid.tile([TC, H], F32)
            h2Ts = [mid.tile([128, TC], BF16, tag=f"h2Ts{j}", name=f"h2Ts{j}")
                    for j in range(KH)]
            h2f8T = [mid.tile([128, TC], F8, tag=f"h2f8T{j}", name=f"h2f8T{j}")
                     for j in range(KH)]

            def rms_scale(xt, d, tag):
                sq = sb.tile([TC, H], F32, tag="rmssq", bufs=1)
                nc.vector.tensor_mul(sq[:, :d], xt, xt)
                red = sb.tile([TC, 1], F32, tag=f"rred{tag}")
                nc.vector.tensor_reduce(red, sq[:, :d], axis=AX.X, op=ALU.add)
                nc.scalar.activation(red, red, AF.Sqrt, bias=eps_sb[:TC], scale=1.0 / d)
                nc.vector.reciprocal(red, red)
                return red

            # ================= ATTENTION PHASE =================
            with tc.tile_pool(name="apool", bufs=2) as ap_, \
                 tc.tile_pool(name="wstream", bufs=3) as wstream:
                x_sb = ap_.tile([TC, H], F32, tag="x_sb", bufs=1)
                nc.sync.dma_start(out=x_sb, in_=x_chunk[:, :])
                rope_q_sb = ap_.tile([TC, 4, DH // 2], F32, tag="ropeq", bufs=1)
                nc.sync.dma_start(out=rope_q_sb, in_=rope_q[:, :, :])
                rope_k_sb = ap_.tile([TC, 4, DH // 2], F32, tag="ropek", bufs=1)
                nc.sync.dma_start(out=rope_k_sb, in_=rope_k[:, :, :])

                rs1 = rms_scale(x_sb, H, "1")
                h1_bf = ap_.tile([TC, H], BF16, tag="h1bf", bufs=1)
                nc.vector.tensor_scalar_mul(h1_bf, x_sb, rs1)

                # h1T via PE transpose
                h1T = []
                for j in range(KH):
                    pt = ps_sm.tile([128, 128], BF16, tag="pstb")
                    nc.tensor.transpose(pt, h1_bf[:, j * 128:(j + 1) * 128], ident_bf)
                    t_ = ap_.tile([128, TC], BF16, tag=f"h1T{j}", bufs=1)
                    nc.vector.tensor_copy(t_, pt)
                    h1T.append(t_)

                # qkv = h1 @ wqkv  -> [TC, 3072] fp32
                qkv_f = ap_.tile([TC, QKV], F32, tag="qkvf", bufs=1)
                for n in (4, 5):
                    wk = wstream.tile([128, 8, 512], BF16, tag="wst")
                    wk2 = wstream.tile([128, 8, 512], BF16, tag="wst")
                    nc.sync.dma_start(
                        out=wk,
                        in_=bass.AP(tensor=wqkv_bf, offset=n * 512,
                                    ap=[[QKV, 128], [128 * QKV, 8], [1, 512]]))
                    nc.scalar.dma_start(
                        out=wk2,
                        in_=bass.AP(tensor=wqkv_bf, offset=8 * 128 * QKV + n * 512,
                                    ap=[[QKV, 128], [128 * QKV, 8], [1, 512]]))
                    pq = ps_big.tile([TC, 512], F32, tag="mm512")
                    for k in range(KH):
                        src = wk[:, k, :] if k < 8 else wk2[:, k - 8, :]
                        nc.tensor.matmul(pq, h1T[k], src,
                                         start=(k == 0), stop=(k == KH - 1))
                    nc.vector.tensor_copy(qkv_f[:, n * 512:(n + 1) * 512], pq)

                q3 = qkv_f[:, 0:NH * DH].rearrange("p (h d) -> p h d", h=NH)
                k3 = qkv_f[:, NH * DH:(NH + NKV) * DH].rearrange(
                    "p (h d) -> p h d", h=NKV)
                v2d = qkv_f[:, (NH + NKV) * DH:]

                # per-head rmsnorm on q, k (in fp32, in place)
                def qk_norm(x3, nh, tag):
                    sq = sb.tile([TC, H], F32, tag="rmssq", bufs=1)
                    x2dv = x3.rearrange("p h d -> p (h d)")
                    nc.vector.tensor_mul(sq[:, :nh * DH], x2dv, x2dv)
                    red = ap_.tile([TC, nh, 1], F32, tag=f"qred{tag}")
                    nc.vector.tensor_reduce(
                        red, sq[:, :nh * DH].rearrange("p (h d) -> p h d", h=nh),
                        axis=AX.X, op=ALU.add)
                    nc.scalar.activation(red.rearrange("p h one -> p (h one)"), red.rearrange("p h one -> p (h one)"), AF.Sqrt, bias=eps_sb[:TC], scale=1.0 / DH)
                    nc.vector.reciprocal(
                        red.rearrange("p h one -> p (h one)"),
                        red.rearrange("p h one -> p (h one)"))
                    for h in range(nh):
                        nc.vector.tensor_scalar_mul(
                            x3[:, h, :], x3[:, h, :], red[:, h, :])

                qk_norm(k3, NKV, "k")

                # rope (+ qk-norm weight folded into tables), cast into qkv_bf
                qkv_bf = ap_.tile([TC, QKV], BF16, tag="qkvbf", bufs=1)
                qbf3 = qkv_bf[:, 0:NH * DH].rearrange("p (h d) -> p h d", h=NH)
                kbf3 = qkv_bf[:, NH * DH:(NH + NKV) * DH].rearrange(
                    "p (h d) -> p h d", h=NKV)

                def rope(x3, obf3, nh, tab):
                    c1 = _bc(tab[:, 0, :], nh)
                    s1 = _bc(tab[:, 1, :], nh)
                    c2 = _bc(tab[:, 2, :], nh)
                    s2 = _bc(tab[:, 3, :], nh)
                    x1 = x3[:, :, 0:DH // 2]
                    x2 = x3[:, :, DH // 2:DH]
                    t1 = ap_.tile([TC, NH, DH // 2], F32, tag="rp1", bufs=1)
                    tn = ap_.tile([TC, NH, DH // 2], F32, tag="rpn", bufs=1)
                    t1v = t1[:, :nh, :]
                    tnv = tn[:, :nh, :]
                    nc.vector.tensor_mul(t1v, x1, c1)
                    nc.vector.tensor_mul(tnv, x2, s1)
                    nc.vector.tensor_sub(t1v, t1v, tnv)
                    nc.vector.tensor_copy(obf3[:, :, 0:DH // 2], t1v)
                    nc.vector.tensor_mul(t1v, x2, c2)
                    nc.vector.tensor_mul(tnv, x1, s2)
                    nc.vector.tensor_add(t1v, t1v, tnv)
                    nc.vector.tensor_copy(obf3[:, :, DH // 2:DH], t1v)

                rope(k3, kbf3, NKV, rope_k_sb)
                nc.vector.tensor_copy(qkv_bf[:, (NH + NKV) * DH:], v2d)

                # transpose k heads -> kT_in (DRAM)
                for g in range(NKV):
                    pt = ps_sm.tile([128, 128], BF16, tag="pstb")
                    nc.tensor.transpose(
                        pt, qkv_bf[:, (NH + g) * DH:(NH + g + 1) * DH],
                        ident_bf)
                    t_ = ap_.tile([DH, TC], BF16, tag="kTs")
                    nc.vector.tensor_copy(t_, pt)
                    nc.sync.dma_start(
                        out=bass.AP(tensor=kv_in.tensor,
                                    offset=kv_in.offset + g * DH * TC,
                                    ap=[[TC, DH], [1, TC]]),
                        in_=t_)
                nc.sync.dma_start(
                    out=bass.AP(tensor=kv_in.tensor,
                                offset=kv_in.offset + NKV * DH * TC,
                                ap=[[NKV * DH, TC], [1, NKV * DH]]),
                    in_=qkv_bf[:, (NH + NKV) * DH:])
                nc.gpsimd.collective_compute(
                    "AllGather", ALU.bypass, replica_groups=rg,
                    ins=[kv_in.opt()], outs=[kv_out.opt()])

                # q columns of the projection (overlaps the kv AllGather)
                for n in range(4):
                    wk = wstream.tile([128, 8, 512], BF16, tag="wst")
                    wk2 = wstream.tile([128, 8, 512], BF16, tag="wst")
                    nc.sync.dma_start(
                        out=wk,
                        in_=bass.AP(tensor=wqkv_bf, offset=n * 512,
                                    ap=[[QKV, 128], [128 * QKV, 8], [1, 512]]))
                    nc.scalar.dma_start(
                        out=wk2,
                        in_=bass.AP(tensor=wqkv_bf, offset=8 * 128 * QKV + n * 512,
                                    ap=[[QKV, 128], [128 * QKV, 8], [1, 512]]))
                    pq = ps_big.tile([TC, 512], F32, tag="mm512")
                    for k in range(KH):
                        src = wk[:, k, :] if k < 8 else wk2[:, k - 8, :]
                        nc.tensor.matmul(pq, h1T[k], src,
                                         start=(k == 0), stop=(k == KH - 1))
                    nc.vector.tensor_copy(qkv_f[:, n * 512:(n + 1) * 512], pq)
                qk_norm(q3, NH, "q")
                rope(q3, qbf3, NH, rope_q_sb)
                # transpose q heads -> qT
                qT = []
                for h in range(NH):
                    pt = ps_sm.tile([128, 128], BF16, tag="pstb")
                    nc.tensor.transpose(
                        pt, qkv_bf[:, h * DH:(h + 1) * DH], ident_bf)
                    t_ = ap_.tile([DH, TC], BF16, tag=f"qT{h}", bufs=1)
                    nc.vector.tensor_copy(t_, pt)
                    qT.append(t_)

                # attention per q head; kT/v loaded per kv-head group.
                # software-pipelined: scores/exp for head h are emitted before
                # the ctx-tail of head h-1 so the in-order PE queue overlaps
                # PE scores with the DVE mask-add / Act exp of the prior head.
                ctxT = [None] * NH

                def load_group(g):
                    kT = ap_.tile([DH, N_CORES, TC], BF16, tag="kTg", bufs=2)
                    nc.scalar.dma_start(
                        out=kT,
                        in_=bass.AP(
                            tensor=kv_out.tensor,
                            offset=kv_out.offset + g * DH * TC,
                            ap=[[TC, DH], [KVSZ, N_CORES], [1, TC]]))
                    vg = [ap_.tile([TC, DH + 1], BF16, tag=f"vg{j}", bufs=2,
                                   name=f"vg{j}")
                          for j in range(N_CORES)]
                    for j in range(N_CORES):
                        nc.vector.memset(vg[j][:, DH:DH + 1], 1.0)
                        nc.sync.dma_start(
                            out=vg[j][:, 0:DH],
                            in_=bass.AP(
                                tensor=kv_out.tensor,
                                offset=kv_out.offset + j * KVSZ
                                + NKV * DH * TC + g * DH,
                                ap=[[NKV * DH, TC], [1, DH]]))
                    return kT, vg

                def head_front(h, kT, vg):
                    probs = ap_.tile([128, N_CORES, TC], F32, tag="probs",
                                     bufs=2)
                    for half in range(2):
                        ps = ps_big.tile([TC, 512], F32, tag="mm512")
                        for jj in range(4):
                            j = half * 4 + jj
                            nc.tensor.matmul(
                                ps[:, jj * TC:(jj + 1) * TC],
                                kT[:, j, :], qT[h], start=True, stop=True)
                        nc.vector.tensor_add(
                            probs.rearrange("p j q -> p (j q)")
                            [:, half * 512:(half + 1) * 512],
                            ps,
                            mask_sb.rearrange("p j q -> p (j q)")
                            [:, half * 512:(half + 1) * 512])
                    pflat = probs.rearrange("p j q -> p (j q)")
                    probs_bf = ap_.tile([128, N_CORES, TC], BF16, tag="probsbf",
                                        bufs=2)
                    nc.scalar.activation(
                        probs_bf.rearrange("p j q -> p (j q)"), pflat,
                        AF.Exp, scale=SCALE)
                    return probs_bf

                def head_tail(h, probs_bf, vg):
                    pctx_t = ps_big.tile([TC, 512], F32, tag="mm512")
                    pctx = pctx_t[:, 0:DH + 1]
                    for j in range(N_CORES):
                        nc.tensor.matmul(pctx, probs_bf[:, j, :], vg[j],
                                         start=(j == 0),
                                         stop=(j == N_CORES - 1))
                    rden = sb.tile([TC, 1], F32, tag="rden")
                    nc.vector.reciprocal(rden, pctx[:, DH:DH + 1])
                    ctx_bf = sb.tile([TC, DH], BF16, tag="ctxbf")
                    nc.vector.tensor_scalar_mul(ctx_bf, pctx[:, 0:DH], rden)
                    pt2 = ps_sm.tile([128, 128], BF16, tag="pstb")
                    nc.tensor.transpose(pt2, ctx_bf, ident_bf)
                    t_ = ap_.tile([DH, TC], BF16, tag=f"ctxT{h}", bufs=1)
                    nc.scalar.activation(t_, pt2, AF.Copy)
                    ctxT[h] = t_

                GSZ = NH // NKV
                cur = load_group(0)
                nxt = None
                pend = None
                for h in range(NH):
                    if h % GSZ == 0 and h > 0:
                        cur = nxt
                    if h % GSZ == 1 and h // GSZ < NKV - 1:
                        nxt = load_group(h // GSZ + 1)
                    probs_bf = head_front(h, cur[0], cur[1])
                    if pend is not None:
                        head_tail(*pend)
                    pend = (h, probs_bf, cur[1])
                head_tail(*pend)

                # attn_out = ctx @ wo ; x2 = x + attn_out. The x2 square-sum
                # for rmsnorm is accumulated per 512-col tile so the h2 chain
                # (and the h2 AllGather) starts right after the last wo tile.
                red4 = sb.tile([TC, H // 512], F32, tag="red4", bufs=1)
                for n in range(H // 512):
                    wk = wstream.tile([128, 8, 512], BF16, tag="wst")
                    wk2 = wstream.tile([128, 8, 512], BF16, tag="wst")
                    nc.sync.dma_start(
                        out=wk,
                        in_=bass.AP(tensor=wo_bf, offset=n * 512,
                                    ap=[[H, 128], [128 * H, 8], [1, 512]]))
                    nc.scalar.dma_start(
                        out=wk2,
                        in_=bass.AP(tensor=wo_bf, offset=8 * 128 * H + n * 512,
                                    ap=[[H, 128], [128 * H, 8], [1, 512]]))
                    po = ps_big.tile([TC, 512], F32, tag="mm512")
                    for k in range(NH * DH // 128):
                        src = wk[:, k, :] if k < 8 else wk2[:, k - 8, :]
                        nc.tensor.matmul(po, ctxT[k], src,
                                         start=(k == 0), stop=(k == KH - 1))
                    nc.vector.tensor_add(x2_sb[:, n * 512:(n + 1) * 512], po,
                                         x_sb[:, n * 512:(n + 1) * 512])
                    sqp = sb.tile([TC, 512], F32, tag="sqp", bufs=2)
                    nc.vector.tensor_mul(sqp, x2_sb[:, n * 512:(n + 1) * 512],
                                         x2_sb[:, n * 512:(n + 1) * 512])
                    nc.vector.tensor_reduce(red4[:, n:n + 1], sqp, axis=AX.X,
                                            op=ALU.add)

                # ---- h2 (still inside attention pool scope) ----
                rs2 = sb.tile([TC, 1], F32, tag="rred2")
                nc.vector.tensor_reduce(rs2, red4, axis=AX.X, op=ALU.add)
                nc.scalar.activation(rs2, rs2, AF.Sqrt, bias=eps_sb[:TC],
                                     scale=1.0 / H)
                nc.vector.reciprocal(rs2, rs2)
                h2_f = mid.tile([TC, H], F32, tag="h2f")
                nc.vector.tensor_scalar_mul(h2_f, x2_sb, rs2)
                h2_bf = ap_.tile([TC, H], BF16, tag="h2bf", bufs=1)
                nc.vector.tensor_copy(h2_bf, h2_f)

                for j in range(KH):
                    pt = ps_sm.tile([128, 128], BF16, tag="pstb")
                    nc.tensor.transpose(pt, h2_bf[:, j * 128:(j + 1) * 128], ident_bf)
                    nc.vector.tensor_copy(h2Ts[j], pt)
                    nc.vector.tensor_copy(h2f8T[j], h2Ts[j])
                    nc.sync.dma_start(out=h2T_in[j * 128:(j + 1) * 128, :],
                                      in_=h2f8T[j])

                nc.gpsimd.collective_compute(
                    "AllGather", ALU.bypass, replica_groups=rg,
                    ins=[h2T_in.opt()], outs=[h2T_out.opt()])

            # ================= MOE PHASE =================
            moe_ctx = [tc.tile_pool(name="h2Tp", bufs=1),
                       tc.tile_pool(name="wgup", bufs=6),
                       tc.tile_pool(name="wdp", bufs=1)]
            h2Tp, wgup, wdp = [c.__enter__() for c in moe_ctx]
            # prefetch fp8 wd pair tiles (independent of collectives)
            wd_sb = []
            for j in range(I // 256):
                t_ = wdp.tile([128, 2, H], F8, tag=f"wd{j}", name=f"wd{j}")
                (nc.scalar if j % 2 == 0 else nc.sync).dma_start(
                    out=t_,
                    in_=bass.AP(tensor=wd_f8, offset=j * 128 * 4096,
                                ap=[[4096, 128], [2048, 2], [1, 2048]]))
                wd_sb.append(t_)

            # shared expert on OWN token chunk (no AG dependency - fills
            # the h2T AllGather gap)
            with tc.tile_pool(name="wsp", bufs=3) as wsp, \
                 tc.tile_pool(name="wsdp", bufs=1) as wsdp:
                gus_bf = []
                for n in range(2 * IS // 512):
                    wsg = wsp.tile([128, KH, 512], F8, tag="wsg")
                    (nc.sync if n % 2 == 0 else nc.scalar).dma_start(
                        out=wsg,
                        in_=bass.AP(tensor=wsgu_bf, offset=n * 512,
                                    ap=[[2 * IS, 128], [128 * 2 * IS, KH],
                                        [1, 512]]))
                    pgu = ps_big.tile([TC, 512], F32, tag="mm512")
                    for k in range(KH):
                        nc.tensor.matmul(pgu, h2Ts[k], wsg[:, k, :],
                                         start=(k == 0), stop=(k == KH - 1))
                    t_ = sb.tile([TC, 512], BF16, tag="gusbf", bufs=4,
                                 name=f"gus{n}")
                    if n < IS // 512:
                        nc.scalar.activation(t_, pgu, AF.Silu, scale=1.0 / WS)
                    else:
                        nc.scalar.activation(t_, pgu, AF.Copy,
                                             scale=1.0 / (WS * WS))
                    gus_bf.append(t_)
                # acts_own[t, i] = silu(g)*u ; transpose to [IS, TC]
                actsT = []
                for n in range(IS // 512):
                    nc.vector.tensor_mul(gus_bf[n], gus_bf[n],
                                         gus_bf[n + IS // 512])
                    for jj in range(4):
                        i = n * 4 + jj
                        pt = ps_sm.tile([128, 128], BF16, tag="pstb")
                        nc.tensor.transpose(
                            pt, gus_bf[n][:, jj * 128:(jj + 1) * 128], ident_bf)
                        t_ = sb.tile([128, TC], BF16, tag=f"actsT{i}", bufs=1,
                                     name=f"actsT{i}")
                        nc.vector.tensor_copy(t_, pt)
                        actsT.append(t_)
                # stream wsd row-tiles (i outer) into 4 persistent psums
                pshs = [ps_big.tile([TC, 512], F32, tag="mm512",
                                    name=f"psh{n_}")
                        for n_ in range(H // 512)]
                for i in range(IS // 128):
                    t_ = wsdp.tile([128, H], F8, tag="wsd", bufs=3)
                    (nc.sync if i % 2 == 0 else nc.scalar).dma_start(
                        out=t_, in_=wsd_bf[i * 128:(i + 1) * 128, :])
                    for n in range(H // 512):
                        nc.tensor.matmul(pshs[n], actsT[i],
                                         t_[:, n * 512:(n + 1) * 512],
                                         start=(i == 0),
                                         stop=(i == IS // 128 - 1))
                for n in range(H // 512):
                    nc.vector.tensor_add(
                        shared_sb[:, n * 512:(n + 1) * 512], pshs[n],
                        x2_sb[:, n * 512:(n + 1) * 512])

            # fp32 router on own chunk (after the shared expert so its PE
            # transposes don't block the in-order PE stream during the AG gap)
            with tc.tile_pool(name="rtp", bufs=2) as rtp:
                pr = ps_big.tile([TC, E], F32, tag="mm512")
                for j in range(KH):
                    pt = ps_sm.tile([128, 128], F32, tag="pstf", bufs=1)
                    nc.tensor.transpose(pt, h2_f[:, j * 128:(j + 1) * 128],
                                        ident_f)
                    t_ = rtp.tile([128, TC], F32, tag="h2T32")
                    nc.vector.tensor_copy(t_, pt)
                    nc.tensor.matmul(pr, t_, wrT_sb[:, j, :],
                                     start=(j == 0), stop=(j == KH - 1))
                probs8 = sb.tile([TC, E], F32, tag="probs8")
                nc.scalar.activation(probs8, pr, AF.Exp, scale=1.0)
                den8 = sb.tile([TC, 1], F32, tag="den8")
                nc.vector.tensor_reduce(den8, probs8, axis=AX.X, op=ALU.add)
                rden8 = sb.tile([TC, 1], F32, tag="rden8")
                nc.vector.reciprocal(rden8, den8)
                nc.vector.tensor_scalar_mul(probs8, probs8, rden8)
                mx8 = sb.tile([TC, 8], F32, tag="mx8")
                nc.vector.max(out=mx8, in_=probs8)
                s12 = sb.tile([TC, 1], F32, tag="s12")
                nc.vector.tensor_add(s12, mx8[:, 0:1], mx8[:, 1:2])
                rs12 = sb.tile([TC, 1], F32, tag="rs12")
                nc.vector.reciprocal(rs12, s12)
                eq1 = sb.tile([TC, E], F32, tag="eq1")
                nc.vector.tensor_scalar(eq1, probs8, mx8[:, 0:1], None,
                                        op0=ALU.is_equal)
                eq2 = sb.tile([TC, E], F32, tag="eq2")
                nc.vector.tensor_scalar(eq2, probs8, mx8[:, 1:2], None,
                                        op0=ALU.is_equal)
                nc.vector.tensor_add(eq1, eq1, eq2)
                wm = sb.tile([TC, E], F32, tag="wm")
                nc.vector.tensor_mul(wm, probs8, eq1)
                nc.vector.tensor_scalar_mul(wm, wm, rs12)
                nc.gpsimd.dma_start(out=w_in[:, :], in_=wm)
                nc.gpsimd.collective_compute(
                    "AllGather", ALU.bypass, replica_groups=rg,
                    ins=[w_in.opt()], outs=[w_out.opt()])
            # keep the router ahead of the (long-stalling) gu matmuls in
            # every engine queue - the list scheduler would otherwise sink it
            tc.no_sync_barrier()

            # own-expert weight column -> broadcast row. gpsimd DMA queue
            # (sync/scalar queues must stay clear for h2P/wgu loads) and a
            # partition-broadcast DMA instead of PE matmuls.
            wall = sb.tile([128, T // TC, E], F32, tag="wall", bufs=1)
            nc.gpsimd.dma_start(
                out=wall,
                in_=bass.AP(tensor=w_out.tensor, offset=w_out.offset,
                            ap=[[E, 128], [TC * E, T // TC], [1, E]]))
            nc.vector.tensor_mul(wall, wall, _bc(esel_sb, T // TC))
            wcol_all = sb.tile([128, T // TC, 1], F32, tag="wcol", bufs=1)
            nc.vector.tensor_reduce(wcol_all, wall, axis=AX.X, op=ALU.add)
            nc.gpsimd.dma_start(
                out=bass.AP(tensor=wrow_dram.tensor, offset=wrow_dram.offset,
                            ap=[[1, 128], [TC, T // TC]]),
                in_=wcol_all.rearrange("p c one -> p (c one)"))
            nc.gpsimd.dma_start(
                out=w_bcast,
                in_=bass.AP(tensor=wrow_dram.tensor, offset=wrow_dram.offset,
                            ap=[[0, 128], [1, T]]))

            if True:
                # gu^T = wgu^T @ h2 (fp8 DoubleRow, K=256 per matmul). All g
                # tiles first: the u drains need w_bcast, which arrives late
                # (behind the w AllGather) and would stall the psum pipeline.
                order = list(range(I // 128)) + \
                    [i_ + I // 128 for i_ in range(I // 128)]

                def load_wk(mi):
                    wk = wgup.tile([128, KH // 2, 2, 128], F8, tag="wgu")
                    dma_eng = nc.sync if mi % 2 == 0 else nc.scalar
                    dma_eng.dma_start(
                        out=wk,
                        in_=bass.AP(tensor=wgu_f8, offset=mi * 128 * 2048,
                                    ap=[[2048, 128], [256, KH // 2], [128, 2],
                                        [1, 128]]))
                    return wk

                # prefetch the first wgu tiles while the AllGather is in flight
                wk_q = [load_wk(order[i]) for i in range(4)]

                # gathered fp8 h2^T pairs: plane i of pair j holds k-tile 2j+i
                h2P = []
                for j in range(KH // 2):
                    t_ = h2Tp.tile([128, 2, T], F8, tag=f"h2P{j}",
                                   name=f"h2P{j}")
                    for pl in range(2):
                        (nc.sync if pl == 0 else nc.scalar).dma_start(
                            out=t_[:, pl, :],
                            in_=bass.AP(
                                tensor=h2T_out.tensor,
                                offset=h2T_out.offset
                                + (2 * j + pl) * 128 * TC,
                                ap=[[TC, 128], [H * TC, N_CORES], [1, TC]]))
                    h2P.append(t_)

                def gu_tile(idx, mi):
                    wk = wk_q.pop(0)
                    if idx + 4 < len(order):
                        wk_q.append(load_wk(order[idx + 4]))
                    dst = sb.tile([128, T], BF16,
                                  tag="gtmp" if mi < I // 128 else "utmp",
                                  bufs=I // 128 if mi < I // 128 else 2)
                    for n in range(T // 512):
                        pg = ps_big.tile([128, 512], F32, tag="mm512")
                        for j in range(KH // 2):
                            nc.tensor.matmul(
                                pg, wk[:, j, :, :],
                                h2P[j][:, :, n * 512:(n + 1) * 512],
                                start=(j == 0), stop=(j == KH // 2 - 1),
                                perf_mode=PM.DoubleRow)
                        if mi < I // 128:
                            # g: psum = WS*g -> silu(g)
                            nc.scalar.activation(
                                dst[:, n * 512:(n + 1) * 512], pg,
                                AF.Silu, scale=1.0 / WS)
                        else:
                            # u: psum = WS*u; w_bcast = (AS/WS)*w -> AS*u*w
                            nc.vector.tensor_mul(
                                dst[:, n * 512:(n + 1) * 512], pg,
                                w_bcast[:, n * 512:(n + 1) * 512])
                    return dst

                g_ts = [gu_tile(i_, i_) for i_ in range(I // 128)]
                for i_ in range(I // 128):
                    u_t = gu_tile(I // 128 + i_, i_ + I // 128)
                    # split act muls between DVE and the idle Pool engine
                    (nc.vector if i_ % 2 == 0 else nc.gpsimd).tensor_mul(
                        act_pr[i_ // 2][:, i_ % 2, :], g_ts[i_], u_t)

                # routed partial [T, H] = act^T @ wd (fp8 DoubleRow) -> rs_in
                inv_sw = const.tile([128, 1], F32)
                nc.vector.memset(inv_sw, 1.0 / (AS * WS))
                cnt = 0
                for t in range(T // TC):
                    for n in range(H // 512):
                        pd = ps_big.tile([TC, 512], F32, tag="mm512")
                        for j in range(I // 256):
                            nc.tensor.matmul(
                                pd, act_pr[j][:, :, t * TC:(t + 1) * TC],
                                wd_sb[j][:, :, n * 512:(n + 1) * 512],
                                start=(j == 0), stop=(j == I // 256 - 1),
                                perf_mode=PM.DoubleRow)
                        rt = sb.tile([TC, 512], BF16, tag="rt", bufs=4)
                        if cnt % 2 == 0:
                            nc.scalar.activation(rt, pd, AF.Copy,
                                                 scale=1.0 / (AS * WS))
                        else:
                            nc.vector.tensor_scalar_mul(rt, pd, inv_sw[:TC])
                        (nc.sync if cnt % 2 == 0 else nc.scalar).dma_start(
                            out=rs_in[t * TC:(t + 1) * TC,
                                      n * 512:(n + 1) * 512],
                            in_=rt)
                        cnt += 1
                nc.gpsimd.collective_compute(
                    "ReduceScatter", ALU.add, replica_groups=rg,
                    ins=[rs_in.opt()], outs=[rs_out.opt()])

            for c in reversed(moe_ctx):
                c.__exit__(None, None, None)

            moe_bf = sb.tile([TC, H], BF16, tag="moebf", bufs=1)
            moe_f = sb.tile([TC, H], F32, tag="moef", bufs=1)
            for q in range(2):
                sl = slice(q * 1024, (q + 1) * 1024)
                (nc.sync if q == 0 else nc.scalar).dma_start(
                    out=moe_bf[:, sl], in_=rs_out[:, sl])
                nc.vector.tensor_add(moe_f[:, sl], shared_sb[:, sl],
                                     moe_bf[:, sl])
                (nc.sync if q == 0 else nc.scalar).dma_start(
                    out=out_chunk[:, sl], in_=moe_f[:, sl])

    nc.compile()
    return nc


def _prep_inputs(hidden_states, w_ln1, w_ln2, wqkv, q_norm_w, k_norm_w, wo,
                 w_router, w_gu, w_d, ws_gu, ws_d, positions):
    import ml_dtypes
    bf = ml_dtypes.bfloat16
    f8 = ml_dtypes.float8_e4m3

    x = np.asarray(hidden_states, np.float32).reshape(T, H)
    w_ln1 = np.asarray(w_ln1, np.float32)
    w_ln2 = np.asarray(w_ln2, np.float32)
    wqkv_e = (np.asarray(wqkv, np.float32) * w_ln1[:, None]).astype(bf)
    wo_b = np.asarray(wo, np.float32).astype(bf)
    # fp8 DoubleRow-packed expert weights (see build_nc comments)
    wgu_e = (np.asarray(w_gu, np.float32) * w_ln2[None, :, None] * WS)
    wgu_p = wgu_e.reshape(E, KH // 2, 2, 128, 2 * I // 128, 128) \
        .transpose(0, 4, 3, 1, 2, 5).reshape(E, 2 * I // 128, 128, 2048) \
        .astype(f8)
    wd_e = np.asarray(w_d, np.float32) * WS
    wd_p = wd_e.reshape(E, I // 256, 2, 128, H).transpose(0, 1, 3, 2, 4) \
        .reshape(E, I // 256, 128, 4096).astype(f8)
    wsgu_e = (np.asarray(ws_gu, np.float32) * w_ln2[:, None] * WS).astype(f8)
    wsd_b = (np.asarray(ws_d, np.float32) * WS).astype(f8)
    wrT_e = np.ascontiguousarray(
        (np.asarray(w_router, np.float32) * w_ln2[None, :]).T.astype(np.float32))

    pos = np.asarray(positions).astype(np.float64)
    inv_freq = 1.0 / (10000.0 ** (np.arange(0, DH, 2, dtype=np.float64) / DH))
    freqs = pos[:, None] * inv_freq[None, :]          # [T, 64]
    cos = np.cos(freqs).astype(np.float32)
    sin = np.sin(freqs).astype(np.float32)
    qw = np.asarray(q_norm_w, np.float32)
    kw = np.asarray(k_norm_w, np.float32)

    def rope_tab(w):
        # [T, 4, 64]: (cos*w[:64], sin*w[64:], cos*w[64:], sin*w[:64])
        return np.ascontiguousarray(
            np.stack([cos * w[None, :64], sin * w[None, 64:],
                      cos * w[None, 64:], sin * w[None, :64]], axis=1), np.float32)

    rq = rope_tab(qw)
    rk = rope_tab(kw)

    kidx = np.arange(T)
    in_maps = []
    for c in range(N_CORES):
        rows = np.arange(c * TC, (c + 1) * TC)
        mask = np.ascontiguousarray(
            np.where(rows[:, None] >= kidx[None, :], 0.0, NEG)
            .astype(np.float32).T)  # [T(tk), TC(tq)]
        es = np.zeros((1, E), np.float32)
        es[0, c] = AS / WS
        in_maps.append({
            "x_chunk": np.ascontiguousarray(x[c * TC:(c + 1) * TC]),
            "wqkv_bf": wqkv_e,
            "wo_bf": wo_b,
            "wgu_f8": np.ascontiguousarray(wgu_p[c]),
            "wd_f8": np.ascontiguousarray(wd_p[c]),
            "wsgu_bf": wsgu_e,
            "wsd_bf": wsd_b,
            "wrT": wrT_e,
            "rope_q": np.ascontiguousarray(rq[c * TC:(c + 1) * TC]),
            "rope_k": np.ascontiguousarray(rk[c * TC:(c + 1) * TC]),
            "mask_in": mask,
            "esel": es,
        })
    return in_maps


def kernel(**inputs):
    import os
    if "nc" not in _cache:
        _cache["nc"] = build_nc()
    nc = _cache["nc"]
    in_maps = _prep_inputs(**inputs)
    trace = bool(int(os.environ.get("KERNEL_TRACE", "0")))
    res = run_bass_kernel_spmd(nc, in_maps, core_ids=list(range(N_CORES)),
                               trace=trace)
    _cache["last_result"] = res
    out = np.concatenate(
        [res.results[c]["out_chunk"] for c in range(N_CORES)], axis=0)
    return out.reshape(1, T, H).astype(np.float32)


if __name__ == "__main__":
    import reference
    inp = {k: np.asarray(v) for k, v in reference.setup_inputs().items()}
    got = kernel(**inp)
    exp = np.asarray(reference.reference(**reference.setup_inputs()))
    denom = np.abs(exp).max()
    err = np.abs(got - exp).max() / denom
    print("abs max:", denom, "rel err:", err)

